# revision 75
# baseline (speedup 1.0000x reference)
"""Trainium2 Bass kernel for 3D neighborhood attention (sparse_attention).

Problem: q,k [1,40,40,40,48] fp32, rpb [8,3,3,3]; out [1,24,40,40,40].
Per voxel x: logits[h,kk] = scale * <q[x,h,:], k[x+off_kk,h,:]> + rpb[h,kk]
(zero-padded k at boundaries, kk over 3x3x3 offsets), p = softmax over kk,
out[x,h,:] = sum_kk p[h,kk] * off_kk  (constant integer offsets as values).

Sharding: spatial over H (40 -> 8 slabs of 5 rows per core).

Per-core dataflow (all engines busy):
 - partitions = (head h in 8) x (chunk in 16), chunks tile (W,T) into 4x4
   blocks of 10x10; each partition owns a 5x10x10 = 500-token interior plus
   a zero-padded 7x12x12 halo'd copy of k (host-prepared, fp16).
 - DVE: 54 fp16 tensor_mul (2x mode) form all 27 shifted q*k products
   (free-axis window shifts on W-preshifted k copies; the (a,b) merge
   keeps each instruction inside the ISA's 3-free-dim limit).
 - PE pass 1: the d-reduction is a transpose-with-accumulation: per kk
   one whole-bank opener matmul seeds rpb[h,kk] (stride-0 stationary),
   then for each 128-token x-run 6 matmuls (stationary = product slab,
   moving = fp16 identity) accumulate sum_d prod into PSUM transposed
   as [x-run, (h,chunk)].  fp32 PSUM accumulation = exact d-sum.
 - ACT: exp evacuates PSUM -> SBUF e^T tiles (one [128,512] instr per kk).
 - PE pass 2: softmax denominator Z and the three numerators (the values
   are the constant offsets in {-1,0,1}^3, and exp(l+rpb) sums are plain
   +/- accumulations) via matmuls with stationary = e^T tile and moving =
   +I / -I, accumulated over kk in PSUM [(h,chunk), (4,x-run)].
 - DVE: reciprocal_approx_fast + 3 multiplies -> out[h,chunk,(o,x)] fp16.
"""

import numpy as np

import concourse.bass as bass
import concourse.tile as tile
from concourse import bacc, mybir
from concourse.bass_utils import run_bass_kernel_spmd

NH = 8
HD = 6
DIM = NH * HD
KS = 3
NT = KS**3  # 27
SCALE = HD**-0.5
H = W = T = 40
N_CORES = 8
SLAB = H // N_CORES          # 5 rows of H per core
P = 128

CA, CB, CC = SLAB, 10, 10    # chunk interior dims (h-rows, w, t)
KA, KB, KC = CA + 2, CB + 2, CC + 2   # halo'd k block dims (7, 12, 12)
NCW, NCT = W // CB, T // CC  # 4 x 4 chunk grid
NCH = NCW * NCT              # 16 chunks -> 8 heads * 16 chunks = 128
X = CA * CB * CC             # 500 interior tokens per chunk
KX = KA * KB * KC            # 1008 halo'd tokens
# four 128-token x-runs covering the 500-token interior (last overlaps)
XRUNS = [(0, 128), (128, 256), (256, 384), (X - P, X)]

_prog_cache = {}


def _win_ap(kw, d, oi):
    """Shifted-window AP on a W-preshifted halo'd k tile kw
    [128, 6, 7, 10, 12]: [p, dl(3), ab(50), c(10)] with element =
    kw[p, d, 1+oi+a, b, dl+c] where ab = a*10+b.  The (a, b) merge (a
    stride 120 = 10 b-steps of 12) keeps the AP within the ISA's 3 free
    dims while covering a whole (d, oi, oj) slab in one instruction."""
    base = kw[:, d, 1 + oi, 0, 0]
    part = kw.ap[0]  # [partition_stride, 128]
    return bass.AP(
        base.tensor,
        base.offset,
        [part, [1, KS], [KC, CA * CB], [1, CC]],
    )


def _build_program():
    fp32 = mybir.dt.float32
    fp16 = mybir.dt.float16
    nc = bacc.Bacc("TRN2", target_bir_lowering=False, debug=False,
                   num_devices=N_CORES)
    # qa aggregates rpb [0:27], +I/-I [27:283], then per-d blocks of
    # [q_d (500) | kw0_d (840)] so each warmup DMA carries exactly what
    # the next mult needs (the tile scheduler reorders bare dma_starts)
    QOF = 27 + 2 * P
    KWD = KA * CB * KC          # 840
    DBLK = X + KWD              # 1340
    qa = nc.dram_tensor("qa", [P, QOF + HD * DBLK], fp16,
                        kind="ExternalInput").ap()
    kw = nc.dram_tensor("kw", [KS - 1, P, HD, KA, CB, KC], fp16,
                        kind="ExternalInput").ap()
    out = nc.dram_tensor("out", [P, 3, X], fp16, kind="ExternalOutput").ap()

    EXP = mybir.ActivationFunctionType.Exp

    with tile.TileContext(nc) as tc:
        with (
            tc.tile_pool(name="consts", bufs=1) as cpool,
            tc.tile_pool(name="prod", bufs=6) as ppool,
            tc.tile_pool(name="ev", bufs=1) as epool,
            tc.tile_pool(name="rec", bufs=2) as rpool,
            tc.tile_pool(name="outp", bufs=1) as opool,
            tc.psum_pool(name="ps1", bufs=6) as ps1pool,
            tc.psum_pool(name="ps2", bufs=2) as ps2pool,
        ):
            qa_sb = cpool.tile([P, QOF + HD * DBLK], fp16)
            kw_sb = [cpool.tile([P, HD, KA, CB, KC], fp16, name=f"kw{j}")
                     for j in range(1, KS)]
            # group 0's gate loads first, split per-d so the first mults
            # start as soon as their slices land
            nc.sync.dma_start(qa_sb[:, :QOF + DBLK], qa[:, :QOF + DBLK])
            for d in range(1, HD):
                nc.sync.dma_start(
                    qa_sb[:, QOF + d * DBLK:QOF + (d + 1) * DBLK],
                    qa[:, QOF + d * DBLK:QOF + (d + 1) * DBLK])
            for j in range(KS - 1):
                nc.sync.dma_start(kw_sb[j][:], kw[j])
            rpb_sb = qa_sb[:, 0:NT]
            id_sb = qa_sb[:, NT:QOF].rearrange("p (s c) -> p s c", s=2)
            # e^T tiles per kk: [x, (xrun, hc)]
            e_sb = [epool.tile([P, 4, P], fp16, name=f"e{kk}")
                    for kk in range(NT)]
            out_sb = opool.tile([P, 3, X], fp16)

            # warm up the PE clock while input DMAs land: ~60 dummy
            # matmuls ramp the tensor engine to full pstate so the real
            # pass-1 stream starts at speed
            ps2 = ps2pool.tile([P, 4, P], fp32)
            for _ in range(60):
                nc.tensor.matmul(ps2[:, 0], id_sb[:, 0], id_sb[:, 0],
                                 start=True, stop=True)

            # ---- Phase A: products (DVE), d-sum transposes (PE), exp (ACT)
            for oj in (-1, 0, 1):
                for oi in (-1, 0, 1):
                    prod = ppool.tile([P, HD, KS, X], fp16)
                    pv = prod[:].rearrange("p d t (ab c) -> p d t ab c",
                                           ab=CA * CB, c=CC)
                    for d in range(HD):
                        d0 = QOF + d * DBLK
                        q_b = (qa_sb[:, d0:d0 + X]
                               .rearrange("p (ab c) -> p ab c", ab=CA * CB)
                               .unsqueeze(1)
                               .broadcast_to([P, KS, CA * CB, CC]))
                        if oj == -1:  # kw0 lives inside qa (d-interleaved)
                            base = qa_sb[:, d0 + X + (1 + oi) * CB * KC]
                            win = bass.AP(
                                base.tensor, base.offset,
                                [qa_sb[:].ap[0], [1, KS],
                                 [KC, CA * CB], [1, CC]])
                        else:
                            win = _win_ap(kw_sb[oj][:], d, oi)
                        nc.vector.tensor_mul(pv[:, d], win, q_b)

                    for t in range(KS):
                        kk = (oi + 1) * 9 + (oj + 1) * 3 + t
                        ps1 = ps1pool.tile([P, 4, P], fp32)
                        # one whole-tile rpb opener for all 4 x-runs
                        # (stride-0 stationary, 4x-replicated identity rhs)
                        nc.tensor.matmul(
                            ps1[:],
                            rpb_sb[:, kk:kk + 1].broadcast_to([P, P]),
                            id_sb[:, 0:1].broadcast_to([P, 4, P]),
                            start=True, stop=False, skip_group_check=True)
                        for xb, (x0, x1) in enumerate(XRUNS):
                            for d in range(HD):
                                nc.tensor.matmul(
                                    ps1[:, xb], prod[:, d, t, x0:x1],
                                    id_sb[:, 0], start=False,
                                    stop=(d == HD - 1),
                                    skip_group_check=True)
                        nc.scalar.activation(
                            e_sb[kk][:].rearrange("p r c -> p (r c)"),
                            ps1[:].rearrange("p r c -> p (r c)"), EXP)

            # ---- Phase B: softmax-weighted sums (PE) + divide (DVE)
            # accumulation slot o: 0 = Z, 1..3 = numerators for (i, j, l)
            def _coef(kk, o):
                return (1, kk // 9 - 1, (kk // 3) % 3 - 1, kk % 3 - 1)[o]

            for xb, (x0, x1) in enumerate(XRUNS):
                ps2 = ps2pool.tile([P, 4, P], fp32)
                # one accumulation region at a time: interleaved start/stop
                # groups across psum regions give wrong results on HW
                for o in range(4):
                    used = [kk for kk in range(NT) if _coef(kk, o)]
                    for n, kk in enumerate(used):
                        nc.tensor.matmul(
                            ps2[:, o], e_sb[kk][:, xb],
                            id_sb[:, 0 if _coef(kk, o) > 0 else 1],
                            start=(n == 0), stop=(n == len(used) - 1))
                rr = rpool.tile([P, P], fp32)
                nc.vector.reciprocal_approx_fast(rr[:], ps2[:, 0])
                lo = 0 if xb < 3 else 384 - x0  # skip overlap with run 2
                r_b = (rr[:, lo:].unsqueeze(1)
                       .broadcast_to([P, 3, P - lo]))
                nc.vector.tensor_mul(out_sb[:, :, x0 + lo:x1],
                                     ps2[:, 1:4, lo:], r_b)
                nc.sync.dma_start(out[:, :, x0 + lo:x1],
                                  out_sb[:, :, x0 + lo:x1])

    nc.compile()
    return nc


def _host_prep(q, k, rpb):
    q0 = (np.asarray(q[0], np.float32) * SCALE)
    kp = np.pad(np.asarray(k[0], np.float32),
                ((1, 1), (1, 1), (1, 1), (0, 0)))
    rpb_f = np.asarray(rpb, np.float32).reshape(NH, NT)

    # rpbt[p=(h,c), kk] = rpb[h(p), kk]
    rpbt = np.broadcast_to(
        rpb_f[:, None, :], (NH, NCH, NT)).reshape(P, NT)
    ident = np.concatenate([np.eye(P, dtype=np.float32),
                            -np.eye(P, dtype=np.float32)], axis=1)  # [P,256]

    in_maps = []
    for i in range(N_CORES):
        h0 = i * SLAB
        # qv[p=(h, wB*4+tB), d, (a,b,c)]
        qs = q0[h0:h0 + SLAB].reshape(CA, NCW, CB, NCT, CC, NH, HD)
        qv = qs.transpose(5, 1, 3, 6, 0, 2, 4).reshape(P, HD, X)
        # kw[oj+1][p, d, A, b, C]: halo'd in A (h-rows) and C (t), but
        # pre-shifted in b (w) so the mult AP can merge (a, b)
        ks = kp[h0:h0 + KA]  # [7, 42, 42, 48]
        kwa = np.empty((KS, NH, NCW, NCT, HD, KA, CB, KC), np.float32)
        for j in range(KS):
            for wb in range(NCW):
                for tb in range(NCT):
                    blk = ks[:, 10 * wb + j:10 * wb + j + CB,
                             10 * tb:10 * tb + KC]
                    kwa[j, :, wb, tb] = blk.reshape(
                        KA, CB, KC, NH, HD).transpose(3, 4, 0, 1, 2)
        kwa = kwa.reshape(KS, P, HD, KA * CB * KC)
        # per-d interleave [q_d | kw0_d] behind the rpb/identity header
        inter = np.concatenate([qv, kwa[0]], axis=2)        # [P, HD, 1340]
        qa = np.concatenate(
            [rpbt, ident, inter.reshape(P, -1)], axis=1)
        in_maps.append({
            "qa": np.ascontiguousarray(qa, dtype=np.float16),
            "kw": np.ascontiguousarray(
                kwa[1:].reshape(KS - 1, P, HD, KA, CB, KC),
                dtype=np.float16),
        })
    return in_maps


def _assemble(results):
    full = np.empty((NH, 3, H, W, T), np.float32)
    for i in range(N_CORES):
        o = np.asarray(results[i]["out"], np.float32)
        o = o.reshape(NH, NCW, NCT, 3, CA, CB, CC)
        # -> [h, o, a, wB, b, tB, c]
        o = o.transpose(0, 3, 4, 1, 5, 2, 6).reshape(NH, 3, CA, W, T)
        full[:, :, i * SLAB:(i + 1) * SLAB] = o
    return full.reshape(NH * 3, H, W, T)[None]


def _run(q, k, rpb, **spmd_kwargs):
    if "prog" not in _prog_cache:
        _prog_cache["prog"] = _build_program()
    nc = _prog_cache["prog"]
    in_maps = _host_prep(q, k, rpb)
    res = run_bass_kernel_spmd(nc, in_maps, list(range(N_CORES)),
                               **spmd_kwargs)
    return _assemble(res.results), res


def kernel(q, k, rpb):
    out, _ = _run(q, k, rpb)
    return out


# revision 80
# speedup vs baseline: 1.1572x; 1.1572x over previous
"""Trainium2 Bass kernel for 3D neighborhood attention (sparse_attention).

Problem: q,k [1,40,40,40,48] fp32, rpb [8,3,3,3]; out [1,24,40,40,40].
Per voxel x: logits[h,kk] = scale * <q[x,h,:], k[x+off_kk,h,:]> + rpb[h,kk]
(zero-padded k at boundaries, kk over 3x3x3 offsets), p = softmax over kk,
out[x,h,:] = sum_kk p[h,kk] * off_kk  (constant integer offsets as values).

Sharding: spatial over H (40 -> 8 slabs of 5 rows per core).

Per-core dataflow (all engines busy):
 - partitions = (head h in 8) x (chunk in 16), chunks tile (W,T) into 4x4
   blocks of 10x10; each partition owns a 5x10x10 = 500-token interior plus
   a zero-padded 7x12x12 halo'd copy of k (host-prepared, fp16).
 - DVE: 54 fp16 tensor_mul (2x mode) form all 27 shifted q*k products
   (free-axis window shifts on W-preshifted k copies; the (a,b) merge
   keeps each instruction inside the ISA's 3-free-dim limit).
 - PE pass 1: the d-reduction is a transpose-with-accumulation: per kk
   one whole-bank opener matmul seeds rpb[h,kk] (stride-0 stationary),
   then for each 128-token x-run 6 matmuls (stationary = product slab,
   moving = fp16 identity) accumulate sum_d prod into PSUM transposed
   as [x-run, (h,chunk)].  fp32 PSUM accumulation = exact d-sum.
 - ACT: exp evacuates PSUM -> SBUF e^T tiles (one [128,512] instr per kk).
 - PE pass 2: softmax denominator Z and the three numerators (the values
   are the constant offsets in {-1,0,1}^3, and exp(l+rpb) sums are plain
   +/- accumulations) via matmuls with stationary = e^T tile and moving =
   +I / -I, accumulated over kk in PSUM [(h,chunk), (4,x-run)].
 - DVE: reciprocal_approx_fast + 3 multiplies -> out[h,chunk,(o,x)] fp16.
"""

import numpy as np

import concourse.bass as bass
import concourse.tile as tile
from concourse import bacc, mybir
from concourse.bass_utils import run_bass_kernel_spmd

NH = 8
HD = 6
DIM = NH * HD
KS = 3
NT = KS**3  # 27
SCALE = HD**-0.5
H = W = T = 40
N_CORES = 8
SLAB = H // N_CORES          # 5 rows of H per core
P = 128

CA, CB, CC = SLAB, 10, 10    # chunk interior dims (h-rows, w, t)
KA, KB, KC = CA + 2, CB + 2, CC + 2   # halo'd k block dims (7, 12, 12)
NCW, NCT = W // CB, T // CC  # 4 x 4 chunk grid
NCH = NCW * NCT              # 16 chunks -> 8 heads * 16 chunks = 128
X = CA * CB * CC             # 500 interior tokens per chunk
KX = KA * KB * KC            # 1008 halo'd tokens
# four 128-token x-runs covering the 500-token interior (last overlaps)
XRUNS = [(0, 128), (128, 256), (256, 384), (X - P, X)]

_prog_cache = {}


def _win_ap(kw, d, oi):
    """Shifted-window AP on a W-preshifted halo'd k tile kw
    [128, 6, 7, 10, 12]: [p, dl(3), ab(50), c(10)] with element =
    kw[p, d, 1+oi+a, b, dl+c] where ab = a*10+b.  The (a, b) merge (a
    stride 120 = 10 b-steps of 12) keeps the AP within the ISA's 3 free
    dims while covering a whole (d, oi, oj) slab in one instruction."""
    base = kw[:, d, 1 + oi, 0, 0]
    part = kw.ap[0]  # [partition_stride, 128]
    return bass.AP(
        base.tensor,
        base.offset,
        [part, [1, KS], [KC, CA * CB], [1, CC]],
    )


def _build_program():
    fp32 = mybir.dt.float32
    fp16 = mybir.dt.float16
    nc = bacc.Bacc("TRN2", target_bir_lowering=False, debug=False,
                   num_devices=N_CORES)
    # qa aggregates rpb [0:27], +I/-I [27:283], and q (d-major, 6*500)
    # [283:3283] so one DMA covers the whole phase-A warmup dependency set
    QOF = 27 + 2 * P
    qa = nc.dram_tensor("qa", [P, QOF + HD * X], fp16,
                        kind="ExternalInput").ap()
    kw = nc.dram_tensor("kw", [KS, P, HD, KA, CB, KC], fp16,
                        kind="ExternalInput").ap()
    out = nc.dram_tensor("out", [P, 3, X], fp16, kind="ExternalOutput").ap()

    EXP = mybir.ActivationFunctionType.Exp

    with tile.TileContext(nc) as tc:
        with (
            tc.tile_pool(name="consts", bufs=1) as cpool,
            tc.tile_pool(name="prod", bufs=6) as ppool,
            tc.tile_pool(name="ev", bufs=1) as epool,
            tc.tile_pool(name="rec", bufs=2) as rpool,
            tc.tile_pool(name="outp", bufs=1) as opool,
            tc.psum_pool(name="ps1", bufs=6) as ps1pool,
            tc.psum_pool(name="ps2", bufs=2) as ps2pool,
        ):
            qa_sb = cpool.tile([P, QOF + HD * X], fp16)
            kw_sb = [cpool.tile([P, HD, KA, CB, KC], fp16, name=f"kw{j}")
                     for j in range(KS)]
            # group 0's gate loads first, split per-d so the first mults
            # start as soon as their slices land
            nc.sync.dma_start(qa_sb[:, :QOF + X], qa[:, :QOF + X])
            nc.sync.dma_start(kw_sb[0][:, 0], kw[0, :, 0])
            for d in range(1, HD):
                nc.sync.dma_start(qa_sb[:, QOF + d * X:QOF + (d + 1) * X],
                                  qa[:, QOF + d * X:QOF + (d + 1) * X])
                nc.sync.dma_start(kw_sb[0][:, d], kw[0, :, d])
            for j in range(1, KS):
                nc.sync.dma_start(kw_sb[j][:], kw[j])
            rpb_sb = qa_sb[:, 0:NT]
            id_sb = qa_sb[:, NT:QOF].rearrange("p (s c) -> p s c", s=2)
            qv_sb = qa_sb[:, QOF:].rearrange("p (d x) -> p d x", d=HD)
            # e^T tiles per kk: [x, (xrun, hc)]
            e_sb = [epool.tile([P, 4, P], fp16, name=f"e{kk}")
                    for kk in range(NT)]
            out_sb = opool.tile([P, 3, X], fp16)

            # warm up the PE clock while input DMAs land: ~60 dummy
            # matmuls ramp the tensor engine to full pstate so the real
            # pass-1 stream starts at speed
            ps2 = ps2pool.tile([P, 4, P], fp32)
            for _ in range(60):
                nc.tensor.matmul(ps2[:, 0], id_sb[:, 0], id_sb[:, 0],
                                 start=True, stop=True)

            # ---- Phase A: products (DVE), d-sum transposes (PE), exp (ACT)
            for oj in (-1, 0, 1):
                for oi in (-1, 0, 1):
                    prod = ppool.tile([P, HD, KS, X], fp16)
                    pv = prod[:].rearrange("p d t (ab c) -> p d t ab c",
                                           ab=CA * CB, c=CC)
                    for d in range(HD):
                        q_b = (qv_sb[:, d]
                               .rearrange("p (ab c) -> p ab c", ab=CA * CB)
                               .unsqueeze(1)
                               .broadcast_to([P, KS, CA * CB, CC]))
                        nc.vector.tensor_mul(
                            pv[:, d], _win_ap(kw_sb[oj + 1][:], d, oi), q_b)

                    for t in range(KS):
                        kk = (oi + 1) * 9 + (oj + 1) * 3 + t
                        ps1 = ps1pool.tile([P, 4, P], fp32)
                        # one whole-tile rpb opener for all 4 x-runs
                        # (stride-0 stationary, 4x-replicated identity rhs)
                        nc.tensor.matmul(
                            ps1[:],
                            rpb_sb[:, kk:kk + 1].broadcast_to([P, P]),
                            id_sb[:, 0:1].broadcast_to([P, 4, P]),
                            start=True, stop=False, skip_group_check=True)
                        for xb, (x0, x1) in enumerate(XRUNS):
                            for d in range(HD):
                                nc.tensor.matmul(
                                    ps1[:, xb], prod[:, d, t, x0:x1],
                                    id_sb[:, 0], start=False,
                                    stop=(d == HD - 1),
                                    skip_group_check=True)
                        nc.scalar.activation(
                            e_sb[kk][:].rearrange("p r c -> p (r c)"),
                            ps1[:].rearrange("p r c -> p (r c)"), EXP)

            # ---- Phase B: softmax-weighted sums (PE) + divide (DVE)
            # accumulation slot o: 0 = Z, 1..3 = numerators for (i, j, l)
            def _coef(kk, o):
                return (1, kk // 9 - 1, (kk // 3) % 3 - 1, kk % 3 - 1)[o]

            for xb, (x0, x1) in enumerate(XRUNS):
                ps2 = ps2pool.tile([P, 4, P], fp32)
                # one accumulation region at a time: interleaved start/stop
                # groups across psum regions give wrong results on HW
                for o in range(4):
                    used = [kk for kk in range(NT) if _coef(kk, o)]
                    for n, kk in enumerate(used):
                        nc.tensor.matmul(
                            ps2[:, o], e_sb[kk][:, xb],
                            id_sb[:, 0 if _coef(kk, o) > 0 else 1],
                            start=(n == 0), stop=(n == len(used) - 1))
                rr = rpool.tile([P, P], fp32)
                nc.vector.reciprocal_approx_fast(rr[:], ps2[:, 0])
                lo = 0 if xb < 3 else 384 - x0  # skip overlap with run 2
                r_b = (rr[:, lo:].unsqueeze(1)
                       .broadcast_to([P, 3, P - lo]))
                nc.vector.tensor_mul(out_sb[:, :, x0 + lo:x1],
                                     ps2[:, 1:4, lo:], r_b)
                nc.sync.dma_start(out[:, :, x0 + lo:x1],
                                  out_sb[:, :, x0 + lo:x1])

    nc.compile()
    return nc


def _host_prep(q, k, rpb):
    q0 = (np.asarray(q[0], np.float32) * SCALE)
    kp = np.pad(np.asarray(k[0], np.float32),
                ((1, 1), (1, 1), (1, 1), (0, 0)))
    rpb_f = np.asarray(rpb, np.float32).reshape(NH, NT)

    # rpbt[p=(h,c), kk] = rpb[h(p), kk]
    rpbt = np.broadcast_to(
        rpb_f[:, None, :], (NH, NCH, NT)).reshape(P, NT)
    ident = np.concatenate([np.eye(P, dtype=np.float32),
                            -np.eye(P, dtype=np.float32)], axis=1)  # [P,256]

    in_maps = []
    for i in range(N_CORES):
        h0 = i * SLAB
        # qv[p=(h, wB*4+tB), d, (a,b,c)]
        qs = q0[h0:h0 + SLAB].reshape(CA, NCW, CB, NCT, CC, NH, HD)
        qv = qs.transpose(5, 1, 3, 6, 0, 2, 4).reshape(P, HD * X)
        qa = np.concatenate([rpbt, ident, qv], axis=1)
        # kw[oj+1][p, d, A, b, C]: halo'd in A (h-rows) and C (t), but
        # pre-shifted in b (w) so the mult AP can merge (a, b)
        ks = kp[h0:h0 + KA]  # [7, 42, 42, 48]
        kwa = np.empty((KS, NH, NCW, NCT, HD, KA, CB, KC), np.float32)
        for j in range(KS):
            for wb in range(NCW):
                for tb in range(NCT):
                    blk = ks[:, 10 * wb + j:10 * wb + j + CB,
                             10 * tb:10 * tb + KC]
                    kwa[j, :, wb, tb] = blk.reshape(
                        KA, CB, KC, NH, HD).transpose(3, 4, 0, 1, 2)
        kwa = kwa.reshape(KS, P, HD, KA, CB, KC)
        in_maps.append({
            "qa": np.ascontiguousarray(qa, dtype=np.float16),
            "kw": np.ascontiguousarray(kwa, dtype=np.float16),
        })
    return in_maps


def _assemble(results):
    full = np.empty((NH, 3, H, W, T), np.float32)
    for i in range(N_CORES):
        o = np.asarray(results[i]["out"], np.float32)
        o = o.reshape(NH, NCW, NCT, 3, CA, CB, CC)
        # -> [h, o, a, wB, b, tB, c]
        o = o.transpose(0, 3, 4, 1, 5, 2, 6).reshape(NH, 3, CA, W, T)
        full[:, :, i * SLAB:(i + 1) * SLAB] = o
    return full.reshape(NH * 3, H, W, T)[None]


def _run(q, k, rpb, **spmd_kwargs):
    if "prog" not in _prog_cache:
        _prog_cache["prog"] = _build_program()
    nc = _prog_cache["prog"]
    in_maps = _host_prep(q, k, rpb)
    res = run_bass_kernel_spmd(nc, in_maps, list(range(N_CORES)),
                               **spmd_kwargs)
    return _assemble(res.results), res


def kernel(q, k, rpb):
    out, _ = _run(q, k, rpb)
    return out
